# revision 40
# baseline (speedup 1.0000x reference)
"""Trainium2 Bass kernel for a dense transformer encoder block.

Sharding (8 cores): sequence-parallel. Core c handles batch b = c//4 and the
512-token query slice q0 = (c%4)*512. Each core computes K/V projections for
its full batch (duplicated across the 4 cores of a batch), attention for its
own queries over all 16 heads, then the FFN for its token slice. No
collectives; the host pre-transposes x / the mask and gathers the outputs.

On-chip layouts: projections and FFN mm1 run feature-major (contraction dim on
partitions); attention scores are computed k-major ([k_tokens | q] tiles) so
softmax-normalization folds into the AV matmul via an appended ones-column on
V; LayerNorms run token-major using bn_stats/bn_aggr. Matmul operands are
bf16 (same PE rate as fp32r at free dims >= 256, half the DMA/SBUF cost);
accumulation, residuals and LayerNorms stay fp32. The softmax mask-add is
split between the DVE and the otherwise-idle Pool/GPSIMD engine; attention
head outputs stay in SBUF (packed two heads per 128-partition tile) for the
O-projection instead of bouncing through DRAM.
"""

import sys
from contextlib import ExitStack

import numpy as np

for _p in ("/opt/trn_rl_repo", "/opt/pypackages"):
    if _p not in sys.path:
        sys.path.append(_p)

import ml_dtypes  # noqa: E402
import concourse.bass as bass  # noqa: E402
import concourse.tile as tile  # noqa: E402
from concourse import bacc, mybir  # noqa: E402
from concourse.masks import make_identity  # noqa: E402

F32 = mybir.dt.float32
F32R = mybir.dt.float32r
BF16 = mybir.dt.bfloat16
AF = mybir.ActivationFunctionType
ALU = mybir.AluOpType

P = 128
DH = 64            # head dim (fixed)
DFH = DH + 1       # head dim + ones column
LN_EPS = 1e-5
NEG = -1e30

FULL_CFG = dict(B=2, L=2048, D=1024, H=16, DFF=4096, NCORES=8)

# matmul operand dtype (bf16: full PE rate, half DMA/SBUF vs fp32)
MM_DT = BF16


def _mm(nc, out, lhsT, rhs, start, stop):
    nc.tensor.matmul(out, lhsT, rhs, start=start, stop=stop)


def build_bass(cfg):
    B, L, D, H, DFF = cfg["B"], cfg["L"], cfg["D"], cfg["H"], cfg["DFF"]
    NCORES = cfg["NCORES"]
    CPB = NCORES // B          # cores per batch
    TOK = L // CPB             # tokens per core
    KC = D // P                # contraction chunks over D
    KT = L // P                # key-token 128-chunks
    HPQ = min(4, H)            # heads per attention phase
    NQ = H // HPQ
    NTQ = TOK // P             # query-token 128-chunks per core
    DC = min(512, D)           # column chunk for D-wide outputs
    NDC = D // DC
    FQ = min(512, DFF)         # w1 column block
    NFQ = DFF // FQ
    BNF = min(512, D)          # bn_stats subgroup
    NBN = D // BNF
    assert H * DH == D and TOK % P == 0 and L % 512 == 0

    nc = bacc.Bacc(None, target_bir_lowering=False, debug=False)
    with tile.TileContext(nc) as tc, ExitStack() as top, \
            nc.allow_low_precision(reason="fp32r operands for full-rate PE"):
        dram = top.enter_context(tc.tile_pool(name="dram", bufs=1, space="DRAM"))

        def din(name, shape, dtype=F32):
            return dram.tile(shape, dtype, kind="ExternalInput", name=name,
                             uniquify=False)

        # x[b].T with token axis rotated so this core's query slice sits
        # in columns 0:TOK (keys are consistently rotated in mask/V too)
        xt_d = din("xt", [D, L], MM_DT)
        xq_d = din("xq", [TOK, D])        # x[b, q0:q0+TOK]
        mt_d = din("mt", [L, TOK], BF16)  # additive mask, transposed slice
        wq_d = din("wq", [D, D], MM_DT)
        wk_d = din("wk", [D, D], MM_DT)
        wv_d = din("wv", [D, D], MM_DT)
        wo_d = din("wo", [D, D], MM_DT)
        w1_d = din("w1", [D, DFF], MM_DT)
        w2_d = din("w2", [DFF, D], MM_DT)
        bq_d = din("bq", [D])
        bk_d = din("bk", [D])
        vb_d = din("vb", [H * DFH], BF16)  # per-head [bv_h, 0]
        bo_d = din("bo", [D])
        b1_d = din("b1", [DFF])
        b2_d = din("b2", [D])
        g1_d = din("g1", [D])
        be1_d = din("be1", [D])
        g2_d = din("g2", [D])
        be2_d = din("be2", [D])
        out_d = dram.tile([TOK, D], F32, kind="ExternalOutput", name="out",
                          uniquify=False)

        def bcast_row(src_ap):
            # DRAM [n] row -> AP broadcasting to P partitions
            return bass.AP(tensor=src_ap.tensor, offset=src_ap.offset,
                           ap=[[0, P]] + [list(a) for a in src_ap.ap])

        const = top.enter_context(tc.tile_pool(name="const", bufs=1))
        ident = const.tile([P, P], F32, name="ident")
        make_identity(nc, ident)
        ones65 = const.tile([DFH, DH], F32, name="ones65")
        nc.vector.memset(ones65[:], 1.0)
        eps_t = const.tile([P, 1], F32, name="eps_t")
        nc.vector.memset(eps_t[:], LN_EPS)
        bq_sb = const.tile([P, KC], F32, name="bq_sb")
        nc.sync.dma_start(out=bq_sb, in_=bq_d[:].rearrange("(c p) -> p c", p=P))
        bk_sb = const.tile([P, KC], F32, name="bk_sb")
        nc.sync.dma_start(out=bk_sb, in_=bk_d[:].rearrange("(c p) -> p c", p=P))
        b1_sb = const.tile([P, DFF // P], F32, name="b1_sb")
        nc.sync.dma_start(out=b1_sb, in_=b1_d[:].rearrange("(c p) -> p c", p=P))
        ones_c = const.tile([P, H, 1], F32, name="ones_c")
        nc.vector.memset(ones_c[:], 1.0)
        vb_bc = const.tile([P, H, DFH], BF16, name="vb_bc")
        nc.sync.dma_start(
            out=vb_bc,
            in_=bcast_row(vb_d[:].rearrange("(h d) -> h d", d=DFH)))

        # attention head outputs, packed 2 heads per tile, kept in SBUF
        # across the attention -> O-projection boundary
        po = top.enter_context(tc.tile_pool(name="oT_sb", bufs=1))
        oT2 = [po.tile([P, TOK], MM_DT, name=f"oT2_{p}")
               for p in range(H // 2)]
        # O-projection weights live below the attention pools so their DMA
        # (issued at the end of phase A) overlaps the softmax pipeline
        wop = [po.tile([P, D], MM_DT, name=f"wop{hp}")
               for hp in range(H // 2)]

        # ---------------- attention (projections + scores + AV) ------------
        # Phase A: K/V/Q projections for ALL heads up front — one dense PE
        # burst through a single 2-bank PSUM accumulation tag. Phase B: one
        # 16-head softmax/AV pipeline (PE scores+AV, mask-add split DVE/Pool,
        # exp pair-grouped on Act). The in-order PE queue naturally runs A
        # before B; B's DVE/Act/Pool work overlaps A's PE tail.
        with ExitStack() as attn:
            pa = attn.enter_context(tc.tile_pool(name="attn_sb", bufs=1))
            pwork = attn.enter_context(tc.tile_pool(name="attn_wk", bufs=3))

            # phase-A-only tensors (x transposes + projection weights) live
            # in their own pool, closed before phase B so the freed SBUF can
            # hold prefetched tail weights during the softmax pipeline
            px_stack = ExitStack()
            px = px_stack.enter_context(tc.tile_pool(name="px", bufs=1))
            xt = []
            xt_r = xt_d[:].rearrange("(c p) l -> c p l", p=P)
            for kc in range(KC):
                t = px.tile([P, L], MM_DT, name=f"xt{kc}", tag=f"xt{kc}")
                nc.sync.dma_start(out=t, in_=xt_r[kc])
                xt.append(t)

            wq_r = wq_d[:].rearrange("(c p) n -> p c n", p=P)
            wk_r = wk_d[:].rearrange("(c p) n -> p c n", p=P)
            wv_r = wv_d[:].rearrange("(c p) n -> p c n", p=P)

            NP = H // 2            # head pairs
            ktqA = [pa.tile([P, L], MM_DT, name=f"ktq{i}", tag=f"ktq{i}")
                    for i in range(NP)]
            qtqA = [pa.tile([P, TOK], MM_DT, name=f"qtq{i}", tag=f"qtq{i}")
                    for i in range(NP)]
            vaugA = [pa.tile([P, H, DFH], MM_DT, name=f"vaug{t}",
                             tag=f"vaug{t}") for t in range(KT)]
            for t in range(KT):
                nc.vector.tensor_copy(vaugA[t][:, :, DH:DFH], ones_c)

            # Phase-A PSUM pool (2 accumulation banks), closed before phase
            # B so the softmax pipeline gets 6 banks of score tiles
            psA = attn.enter_context(tc.tile_pool(name="psA", bufs=1,
                                                  space="PSUM"))
            psB = psA
            # K projection, feature-major, scaled by 1/8, +bias; one PSUM
            # accumulation window per (pair, 512-token group)
            for i in range(NP):
                wkt = px.tile([P, KC, P], MM_DT, name="wkt", tag="wkt", bufs=2)
                nc.sync.dma_start(out=wkt,
                                  in_=wk_r[:, :, i * P:(i + 1) * P])
                for tg in range(L // 512):
                    kp = psA.tile([P, 512], F32, name="kp", tag="acc", bufs=2)
                    for kc in range(KC):
                        _mm(nc, kp, wkt[:, kc, :],
                            xt[kc][:, tg * 512:(tg + 1) * 512],
                            start=(kc == 0), stop=(kc == KC - 1))
                    nc.vector.tensor_scalar(
                        ktqA[i][:, tg * 512:(tg + 1) * 512], kp,
                        bk_sb[:, i:i + 1], 0.125, ALU.add, ALU.mult)
            # V projection, token-major, into [V|1] layout, +bias; four
            # heads (256 feature columns) per accumulation window
            for qtr in range(4):
                h4 = slice(qtr * (H // 4), (qtr + 1) * (H // 4))
                wvt = px.tile([P, KC, 256], MM_DT, name="wvt", tag="wvt",
                              bufs=2)
                nc.sync.dma_start(out=wvt,
                                  in_=wv_r[:, :, qtr * 256:(qtr + 1) * 256])
                for tci in range(KT):
                    vp = psA.tile([P, 512], F32, name="vp", tag="acc", bufs=2)
                    for kc in range(KC):
                        _mm(nc, vp[:, 0:256],
                            xt[kc][:, tci * P:(tci + 1) * P],
                            wvt[:, kc, :],
                            start=(kc == 0), stop=(kc == KC - 1))
                    nc.vector.tensor_tensor(
                        vaugA[tci][:, h4, 0:DH],
                        vp[:, 0:256].rearrange("p (h d) -> p h d", d=DH),
                        vb_bc[:, h4, 0:DH], ALU.add)
            # Q projection per pair
            for i in range(NP):
                wqt = px.tile([P, KC, P], MM_DT, name="wqt", tag="wqt", bufs=2)
                nc.sync.dma_start(out=wqt,
                                  in_=wq_r[:, :, i * P:(i + 1) * P])
                qp = psA.tile([P, TOK], F32, name="qp", tag="acc", bufs=2)
                for kc in range(KC):
                    _mm(nc, qp, wqt[:, kc, :], xt[kc][:, 0:TOK],
                        start=(kc == 0), stop=(kc == KC - 1))
                nc.vector.tensor_scalar(
                    qtqA[i], qp, bq_sb[:, i:i + 1], None, ALU.add)

            for hp in range(H // 2):
                nc.sync.dma_start(out=wop[hp],
                                  in_=wo_d[hp * P:(hp + 1) * P, :])
            px_stack.close()

            # additive mask, loaded during phase-A compute (issued after the
            # projection-weight DMAs so it doesn't delay the first matmuls)
            mt = pa.tile([P, KT, TOK], BF16, name="mt", tag="mt")
            nc.sync.dma_start(out=mt,
                              in_=mt_d[:].rearrange("(t p) q -> p t q", p=P))

            # Phase B: scores + softmax + AV per head. Multiplicative
            # masking: exp() reads the 2-bank score PSUM tile directly on the
            # Act engine (unmasked), then a cheap all-bf16 SBUF multiply by
            # the 0/1 keep-mask runs on DVE (2x 16-bit mode) or Pool (GPSIMD
            # cannot touch PSUM, but SBUF*SBUF is legal there).
            def softmax_av(h, otp, kg, sp2):
                ssb = pwork.tile([P, 2, TOK], MM_DT, name="ssb",
                                 tag="ssb", bufs=6)
                nc.scalar.activation(ssb, sp2, AF.Exp)
                # keep-mask multiply, ~1/4 on the otherwise-idle Pool
                eng = nc.gpsimd if kg % 4 == 1 else nc.vector
                eng.tensor_tensor(ssb, ssb,
                                  mt[:, kg * 2:kg * 2 + 2, :], ALU.mult)
                for j in range(2):
                    kt = kg * 2 + j
                    _mm(nc, otp[0:DFH, :], vaugA[kt][:, h, :],
                        ssb[:, j, :],
                        start=(kt == 0), stop=(kt == KT - 1))

            def normalize(h, otp):
                rt = pwork.tile([DFH, TOK], F32, name="rt", tag="rt",
                                bufs=2)
                nc.vector.reciprocal(rt[DH:DFH, :], otp[DH:DFH, :])
                rb = psB.tile([P, TOK], F32, name="rb", tag="acc",
                              bufs=2)
                nc.tensor.matmul(rb[0:DH, :], ones65[DH:DFH, :],
                                 rt[DH:DFH, :], start=True, stop=True)
                # verifier forbids two PSUM inputs on tensor_tensor:
                # bounce the broadcast through SBUF first
                rbs = pwork.tile([DH, TOK], F32, name="rbs", tag="rbs",
                                 bufs=2)
                nc.vector.tensor_copy(rbs, rb[0:DH, :])
                nc.vector.tensor_tensor(
                    oT2[h // 2][(h % 2) * DH:(h % 2 + 1) * DH, :],
                    otp[0:DH, :], rbs, ALU.mult)

            # software-pipelined: scores run one group ahead of exp/mask/AV,
            # and each head's normalize is deferred into the next head's
            # first-group slot, so the in-order PE queue never waits on the
            # Act/DVE stages.
            prev_norm = None
            for h in range(H):
                i, s = h // 2, h % 2
                base = slice(s * DH, (s + 1) * DH)
                otp = psB.tile([P, TOK], F32, name="otp", tag="otp",
                               bufs=2)
                pending = []
                for kg in range(KT // 2):
                    sp2 = psB.tile([P, 2, TOK], F32, name="sp2", tag="sp2",
                                   bufs=2)
                    for j in range(2):
                        kt = kg * 2 + j
                        _mm(nc, sp2[:, j, :],
                            ktqA[i][base, kt * P:(kt + 1) * P],
                            qtqA[i][base, :], start=True, stop=True)
                    if kg == 0 and prev_norm is not None:
                        normalize(*prev_norm)
                    pending.append((kg, sp2))
                    if len(pending) > 3:
                        softmax_av(h, otp, *pending.pop(0))
                for pn in pending:
                    softmax_av(h, otp, *pn)
                prev_norm = (h, otp)
            normalize(*prev_norm)

        # ---------------- O-projection + LN1 + transpose + FFN --------------
        with ExitStack() as tail:
            pcd = tail.enter_context(tc.tile_pool(name="cd_sb", bufs=1))
            h_t = [pcd.tile([P, D], F32, name=f"h{t}", tag=f"h{t}")
                   for t in range(NTQ)]
            hT = [pcd.tile([P, TOK], MM_DT, name=f"hT{c}", tag=f"hT{c}")
                  for c in range(KC)]
            bo_bc = pcd.tile([P, D], F32, name="bo_bc")
            nc.sync.dma_start(out=bo_bc, in_=bcast_row(bo_d[:]))
            b2_bc = pcd.tile([P, D], F32, name="b2_bc")
            nc.sync.dma_start(out=b2_bc, in_=bcast_row(b2_d[:]))
            g1_bc = pcd.tile([P, D], F32, name="g1_bc")
            nc.sync.dma_start(out=g1_bc, in_=bcast_row(g1_d[:]))
            be1_bc = pcd.tile([P, D], F32, name="be1_bc")
            nc.sync.dma_start(out=be1_bc, in_=bcast_row(be1_d[:]))
            g2_bc = pcd.tile([P, D], F32, name="g2_bc")
            nc.sync.dma_start(out=g2_bc, in_=bcast_row(g2_d[:]))
            be2_bc = pcd.tile([P, D], F32, name="be2_bc")
            nc.sync.dma_start(out=be2_bc, in_=bcast_row(be2_d[:]))

            def layernorm(x_tile, g_bc, b_bc, wk):
                st = wk.tile([P, NBN, 6], F32, name="lnst", tag="lnst")
                xv = x_tile.rearrange("p (s f) -> p s f", f=BNF)
                for sg in range(NBN):
                    nc.vector.bn_stats(out=st[:, sg, :], in_=xv[:, sg, :])
                mv = wk.tile([P, 2], F32, name="lnmv", tag="lnmv")
                nc.vector.bn_aggr(out=mv, in_=st)
                sq = wk.tile([P, 1], F32, name="lnsq", tag="lnsq")
                nc.scalar.activation(sq, mv[:, 1:2], AF.Sqrt, bias=eps_t)
                nc.vector.reciprocal(sq, sq)
                nc.vector.tensor_scalar(x_tile, x_tile, mv[:, 0:1], sq,
                                        ALU.subtract, ALU.mult)
                nc.vector.tensor_tensor(x_tile, x_tile, g_bc, ALU.mult)
                nc.vector.tensor_tensor(x_tile, x_tile, b_bc, ALU.add)

            with ExitStack() as cph:
                pcwk = cph.enter_context(tc.tile_pool(name="c_wk", bufs=3))
                pcp = cph.enter_context(tc.tile_pool(name="c_ps", bufs=1,
                                                     space="PSUM"))
                xq = []
                xq_r = xq_d[:].rearrange("(t p) d -> t p d", p=P)
                for t in range(NTQ):
                    tl = pcwk.tile([P, D], F32, name=f"xq{t}", tag=f"xq{t}",
                                   bufs=1)
                    nc.sync.dma_start(out=tl, in_=xq_r[t])
                    xq.append(tl)
                # O-projection per token tile (dc inner, prefetched weights)
                # so each tile's residual + LN1 + transposes overlap the next
                # tile's matmuls
                for t in range(NTQ):
                    op = pcp.tile([P, NDC, DC], F32, name=f"op{t}",
                                  tag=f"op{t % 2}", bufs=1)
                    for dc in range(NDC):
                        for hp in range(H // 2):
                            _mm(nc, op[:, dc, :],
                                oT2[hp][:, t * P:(t + 1) * P],
                                wop[hp][:, dc * DC:(dc + 1) * DC],
                                start=(hp == 0), stop=(hp == H // 2 - 1))
                    nc.vector.tensor_tensor(
                        h_t[t], op.rearrange("p d c -> p (d c)"), xq[t],
                        ALU.add)
                    nc.vector.tensor_tensor(h_t[t], h_t[t], bo_bc, ALU.add)
                    layernorm(h_t[t], g1_bc, be1_bc, pcwk)
                    for c in range(KC):
                        tp = pcp.tile([P, P], F32, name="tpp", tag="tpp",
                                      bufs=2)
                        nc.tensor.transpose(tp, h_t[t][:, c * P:(c + 1) * P],
                                            ident)
                        nc.scalar.activation(hT[c][:, t * P:(t + 1) * P], tp,
                                             AF.Copy)

            with ExitStack() as dph:
                pdw = dph.enter_context(tc.tile_pool(name="d_w", bufs=1))
                pdw2 = dph.enter_context(tc.tile_pool(name="d_w2", bufs=3))
                pdp = dph.enter_context(tc.tile_pool(name="d_ps", bufs=1,
                                                     space="PSUM"))
                f1 = [pdw.tile([P, TOK], MM_DT, name=f"f1_{ff}",
                               tag=f"f1_{ff}") for ff in range(DFF // P)]
                for fq in range(NFQ):
                    w1t = [pdw.tile([P, FQ], MM_DT, name=f"w1t{kc}",
                                    tag=f"w1t{kc}", bufs=2)
                           for kc in range(KC)]
                    for kc in range(KC):
                        nc.sync.dma_start(
                            out=w1t[kc],
                            in_=w1_d[kc * P:(kc + 1) * P,
                                     fq * FQ:(fq + 1) * FQ])
                    for ffl in range(FQ // P):
                        ff = fq * (FQ // P) + ffl
                        fp = pdp.tile([P, TOK], F32, name="fp", tag="fp",
                                      bufs=2)
                        for kc in range(KC):
                            _mm(nc, fp, w1t[kc][:, ffl * P:(ffl + 1) * P],
                                hT[kc], start=(kc == 0), stop=(kc == KC - 1))
                        nc.scalar.activation(f1[ff], fp, AF.Relu,
                                             bias=b1_sb[:, ff:ff + 1])
                f2 = [pdw.tile([P, D], F32, name=f"f2_{t}", tag=f"f2_{t}")
                      for t in range(NTQ)]
                for dc in range(NDC):
                    g2p = [pdp.tile([P, DC], F32, name=f"g2p{t}",
                                    tag=f"g2p{t}") for t in range(NTQ)]
                    for kc2 in range(DFF // P):
                        w2t = pdw2.tile([P, DC], MM_DT, name="w2t", tag="w2t",
                                        bufs=6)
                        nc.sync.dma_start(
                            out=w2t,
                            in_=w2_d[kc2 * P:(kc2 + 1) * P,
                                     dc * DC:(dc + 1) * DC])
                        for t in range(NTQ):
                            _mm(nc, g2p[t], f1[kc2][:, t * P:(t + 1) * P], w2t,
                                start=(kc2 == 0), stop=(kc2 == DFF // P - 1))
                    for t in range(NTQ):
                        dsl = slice(dc * DC, (dc + 1) * DC)
                        nc.vector.tensor_tensor(f2[t][:, dsl], g2p[t],
                                                h_t[t][:, dsl], ALU.add)
                        nc.vector.tensor_tensor(f2[t][:, dsl], f2[t][:, dsl],
                                                b2_bc[:, dsl], ALU.add)
                for t in range(NTQ):
                    layernorm(f2[t], g2_bc, be2_bc, pdw2)
                    nc.sync.dma_start(out=out_d[t * P:(t + 1) * P, :],
                                      in_=f2[t])

    nc.compile()
    return nc


def make_in_maps(cfg, inp):
    """Build per-core input dicts from full (host) inputs."""
    B, L, D, H = cfg["B"], cfg["L"], cfg["D"], cfg["H"]
    NCORES = cfg["NCORES"]
    CPB = NCORES // B
    TOK = L // CPB
    f32 = np.float32
    bf16 = ml_dtypes.bfloat16
    x = np.asarray(inp["x"], f32)
    mask = np.asarray(inp["mask"], bool)
    w = {k: np.asarray(inp[k], f32) for k in
         ("wq", "bq", "wk", "bk", "wv", "bv", "wo", "bo", "w1", "b1",
          "w2", "b2", "ln1_g", "ln1_b", "ln2_g", "ln2_b")}
    vb = np.zeros((H, DFH), f32)
    vb[:, 0:DH] = w["bv"].reshape(H, DH)
    shared = dict(wq=w["wq"].astype(bf16), wk=w["wk"].astype(bf16),
                  wv=w["wv"].astype(bf16), wo=w["wo"].astype(bf16),
                  w1=w["w1"].astype(bf16), w2=w["w2"].astype(bf16),
                  bq=w["bq"], bk=w["bk"],
                  vb=vb.reshape(-1).astype(bf16), bo=w["bo"], b1=w["b1"], b2=w["b2"],
                  g1=w["ln1_g"], be1=w["ln1_b"], g2=w["ln2_g"],
                  be2=w["ln2_b"])
    shared = {k: np.ascontiguousarray(v) for k, v in shared.items()}
    in_maps = []
    for c in range(NCORES):
        b, q0 = c // CPB, (c % CPB) * TOK
        xb = x[b]
        xbt = np.ascontiguousarray(xb.T).astype(bf16)
        mt = np.where(mask[b, q0:q0 + TOK, :].T, f32(0.0), f32(1.0))
        m = dict(shared)
        # rotate the token (key) axis so this core's query slice is at
        # columns 0:TOK; keys are consistently reordered in the mask
        m["xt"] = np.ascontiguousarray(np.roll(xbt, -q0, axis=1))
        m["xq"] = np.ascontiguousarray(xb[q0:q0 + TOK])
        m["mt"] = np.ascontiguousarray(np.roll(mt.astype(bf16), -q0, axis=0))
        in_maps.append(m)
    return in_maps


_NC_CACHE = {}
TRACE = False
LAST_RESULTS = None


def _get_nc(key, cfg):
    if key not in _NC_CACHE:
        _NC_CACHE[key] = build_bass(cfg)
    return _NC_CACHE[key]


def kernel(**inputs):
    global LAST_RESULTS
    from concourse.bass_utils import run_bass_kernel_spmd

    cfg = FULL_CFG
    B, L, D = cfg["B"], cfg["L"], cfg["D"]
    NCORES = cfg["NCORES"]
    CPB = NCORES // B
    TOK = L // CPB
    nc = _get_nc("full", cfg)
    in_maps = make_in_maps(cfg, inputs)
    res = run_bass_kernel_spmd(nc, in_maps, core_ids=list(range(NCORES)),
                               trace=TRACE)
    LAST_RESULTS = res
    out = np.empty((B, L, D), np.float32)
    for c in range(NCORES):
        b, q0 = c // CPB, (c % CPB) * TOK
        out[b, q0:q0 + TOK] = res.results[c]["out"]
    return out

